# revision 36
# baseline (speedup 1.0000x reference)
"""Trainium2 Bass kernel for nn_PatchLossOptimizedV4.

loss = mean_b(2*sum_t z2[t]*w_sum[t] - 2*sum_{t,d} wz[t,d]*z[t,d]) / T^2
with w = exp(+sum_ds gt^2 / (2 sigma^2)).

Restructured per batch b:
  A[s,d]  = sum_t w[t,s] * z[t,d]          (PE matmul, w tile is lhsT natively)
  term2_b = sum_{s,d} A[s,d] * z[s,d]
  term1_b = 2 * sum_{t} z2[t] * wsum[t]    (wsum from exp's free accum_out)
  loss_b  = term1_b - 2*term2_b

v2: gt and z are DMA'd in fp16 (halves the DMA-serialization floor;
numpy emulation on the seed-0 data shows rel err ~2e-4, tolerance 2e-2).
Squares + ds-reduction run mostly on DVE in fp16 (2-byte fast modes),
with a tunable share of square chunks on ACT for engine balance.
Sharding: pure data-parallel, 2 batches per core x 8 cores. Each core
returns a [128, 18] partial-sum tile; host reduces in float64.
"""

import math
import sys
from contextlib import ExitStack

import numpy as np

sys.path.insert(0, "/opt/trn_rl_repo")

import concourse.bass as bass
import concourse.bacc as bacc
import concourse.tile as tile
from concourse import mybir
from concourse.bass_utils import run_bass_kernel_spmd

B, T, D, DS = 16, 1024, 256, 4
N_CORES = 8
BPC = B // N_CORES          # batches per core
P = 128                     # partitions
NT = T // P                 # t/s tiles per batch
NCOLS = BPC * (NT + 1)      # accum columns (8 drains + 1 term1 per batch)
F32 = mybir.dt.float32
F16 = mybir.dt.float16
BF16 = mybir.dt.bfloat16
ALU = mybir.AluOpType
ACTF = mybir.ActivationFunctionType

_NC_CACHE = {}

DEF_GT_PLAN = tuple(["gpsimd", "sync"] * (BPC * NT // 2))
SHIFT = 10.5   # w is computed as exp(c*wpre - SHIFT) to fit fp16; host rescales


def build_nc(c_ds, reps=1, n_ch=2, gt_plan=None, z_engine="gpsimd",
             out_engine="gpsimd", act_sq=20, drain_engine="vector",
             z2_engine="vector", shift=SHIFT):
    nc = bacc.Bacc(None, target_bir_lowering=False)
    gt = nc.declare_dram_parameter("gt", [BPC, T, T, DS], F16, isOutput=False)
    z = nc.declare_dram_parameter("z", [BPC, T, D], F16, isOutput=False)
    out = nc.declare_dram_parameter("out", [P, NCOLS], F32, isOutput=True)

    uniform = all(abs(c - c_ds[0]) < 1e-30 for c in c_ds)
    if gt_plan is None:
        gt_plan = DEF_GT_PLAN

    def eng(name):
        return {"sync": nc.sync, "scalar": nc.scalar, "vector": nc.vector,
                "gpsimd": nc.gpsimd}[name]

    # which global square-chunks (of BPC*NT*n_ch) run on ACT instead of DVE
    n_chunks = BPC * NT * n_ch
    act_set = set(int(i * n_chunks / act_sq) for i in range(act_sq)) if act_sq else set()

    with tile.TileContext(nc) as tc, ExitStack() as ctx, \
            nc.allow_low_precision(reason="fp16 pipeline; loss tolerance 2e-2"):
        gt_pool = ctx.enter_context(tc.tile_pool(name="gt", bufs=3))
        sq_pool = ctx.enter_context(tc.tile_pool(name="sq", bufs=2))
        r1_pool = ctx.enter_context(tc.tile_pool(name="r1", bufs=2))
        w_pool = ctx.enter_context(tc.tile_pool(name="w", bufs=2))
        z_pool = ctx.enter_context(tc.tile_pool(name="z", bufs=2))
        small = ctx.enter_context(tc.tile_pool(name="small", bufs=2))
        scratch = ctx.enter_context(tc.tile_pool(name="scratch", bufs=2))
        accum_pool = ctx.enter_context(tc.tile_pool(name="accum", bufs=1))
        psum = ctx.enter_context(tc.tile_pool(name="psum", bufs=1, space="PSUM"))

        accum = accum_pool.tile([P, NCOLS], F32)
        bias_t = accum_pool.tile([P, 1], F32, tag="bias")
        nc.gpsimd.memset(bias_t[:], -float(shift))

        for b in [b for _ in range(reps) for b in range(BPC)]:
            ztile = z_pool.tile([P, NT, D], F16, tag="ztile")
            eng(z_engine).dma_start(out=ztile[:], in_=z[b].rearrange("(i p) d -> p i d", p=P))

            z2 = small.tile([P, NT], F32, tag="z2")
            wsum = small.tile([P, NT], F32, tag="wsum")
            for i in range(NT):
                zsq = scratch.tile([P, D], F16, tag="zsq")
                if z2_engine == "scalar":
                    nc.scalar.activation(
                        out=zsq[:], in_=ztile[:, i, :], func=ACTF.Square,
                        accum_out=z2[:, i : i + 1],
                    )
                else:
                    eng(z2_engine).scalar_tensor_tensor(
                        out=zsq[:], in0=ztile[:, i, :], scalar=1.0,
                        in1=ztile[:, i, :], op0=ALU.mult, op1=ALU.mult,
                        accum_out=z2[:, i : i + 1],
                    )

            psum_tiles = [
                psum.tile([P, D], F32, tag=f"ps{sc}", name=f"ps{sc}")
                for sc in range(NT)
            ]

            for i in range(NT):
                gtt = gt_pool.tile([P, T, DS], F16, tag="gt")
                sq = sq_pool.tile([P, T, DS], F16, tag="sq")
                r1 = r1_pool.tile([P, T, 2], F16, tag="r1")
                wpre = w_pool.tile([P, T], F16, tag="wpre")
                cs = T // n_ch
                tile_idx = b * NT + i
                dma_eng = eng(gt_plan[tile_idx])
                for ch in range(n_ch):
                    sl = slice(ch * cs, (ch + 1) * cs)
                    dma_eng.dma_start(
                        out=gtt[:, sl, :],
                        in_=gt[b, i * P : (i + 1) * P, sl, :],
                    )
                    g_idx = tile_idx * n_ch + ch
                    if uniform:
                        if g_idx in act_set:
                            nc.scalar.square(out=sq[:, sl, :], in_=gtt[:, sl, :])
                        else:
                            nc.vector.tensor_tensor(
                                out=sq[:, sl, :], in0=gtt[:, sl, :],
                                in1=gtt[:, sl, :], op=ALU.mult,
                            )
                    else:
                        for ds in range(DS):
                            nc.scalar.activation(
                                out=sq[:, sl, ds], in_=gtt[:, sl, ds],
                                func=ACTF.Square, scale=math.sqrt(c_ds[ds]),
                            )
                    nc.vector.tensor_tensor(
                        out=r1[:, sl, :], in0=sq[:, sl, 0:2],
                        in1=sq[:, sl, 2:4], op=ALU.add,
                    )
                    nc.vector.tensor_tensor(
                        out=wpre[:, sl], in0=r1[:, sl, 0],
                        in1=r1[:, sl, 1], op=ALU.add,
                    )
                w_t = w_pool.tile([P, T], F16, tag="w")
                nc.scalar.activation(
                    out=w_t[:], in_=wpre[:], func=ACTF.Exp,
                    scale=float(c_ds[0]) if uniform else 1.0,
                    bias=bias_t[:],
                    accum_out=wsum[:, i : i + 1],
                )
                for sc in range(NT):
                    nc.tensor.matmul(
                        out=psum_tiles[sc][:],
                        lhsT=w_t[:, sc * P : (sc + 1) * P],
                        rhs=ztile[:, i, :],
                        start=(i == 0), stop=(i == NT - 1),
                    )

            col0 = b * (NT + 1)
            for sc in range(NT):
                dsc = scratch.tile([P, D], F32, tag="drain")
                eng(drain_engine).scalar_tensor_tensor(
                    out=dsc[:], in0=psum_tiles[sc][:], scalar=-1.0,
                    in1=ztile[:, sc, :], op0=ALU.mult, op1=ALU.mult,
                    accum_out=accum[:, col0 + sc : col0 + sc + 1],
                )
            t1s = scratch.tile([P, NT], F32, tag="t1s")
            nc.vector.scalar_tensor_tensor(
                out=t1s[:], in0=wsum[:], scalar=1.0,
                in1=z2[:], op0=ALU.mult, op1=ALU.mult,
                accum_out=accum[:, col0 + NT : col0 + NT + 1],
            )

        eng(out_engine).dma_start(out=out[:, :], in_=accum[:])
    nc.finalize()
    return nc


def kernel(z: np.ndarray, gt_dT: np.ndarray, sigma: np.ndarray) -> np.ndarray:
    sigma64 = np.asarray(sigma, dtype=np.float64)
    c_ds = tuple(float(c) for c in 1.0 / (2.0 * sigma64 * sigma64))

    key = c_ds
    if key not in _NC_CACHE:
        _NC_CACHE[key] = build_nc(c_ds)
    nc = _NC_CACHE[key]

    z = np.ascontiguousarray(z, dtype=np.float16)
    gt_dT = np.ascontiguousarray(gt_dT, dtype=np.float16)
    in_maps = [
        {"gt": gt_dT[c * BPC : (c + 1) * BPC], "z": z[c * BPC : (c + 1) * BPC]}
        for c in range(N_CORES)
    ]
    res = run_bass_kernel_spmd(nc, in_maps, core_ids=list(range(N_CORES)))

    total = np.float64(0.0)
    for r in res.results:
        total += np.asarray(r["out"], dtype=np.float64).sum()
    # accum holds exp(-SHIFT)*(term1_b/2 - term2_b) summed over this core's
    # batches; loss = mean_b(2*that)/T^2 after undoing the shift
    loss = 2.0 * total * math.exp(SHIFT) / (B * T * T)
    return np.float32(loss)
